# revision 1
# baseline (speedup 1.0000x reference)
"""Sobel gradient magnitude kernel for Trainium2 (8 NeuronCores, batch-sharded).

out = sqrt(gx^2 + gy^2), gx/gy = 3x3 depthwise convs (zero-padded) of
x [16, 64, 256, 256] fp32.

Per-core layout (2 batches x 64 ch = 128 images of 256x256):
  - image rows on partitions, two 128-row halves side by side in the free dim
  - vertical 3-taps as banded-matrix matmuls on TensorE (fp16 in, fp32 psum)
  - horizontal taps folded into PSUM accumulation via output-shifted matmuls
  - PSUM evacuated by ScalarE as Square; GPSIMD adds gx^2+gy^2; ScalarE Sqrt
  - rows 127/128 of each image (cross-half seam) recomputed in one batched
    late pass over all images and scattered over the main output
"""

import os
import numpy as np
from contextlib import ExitStack

import concourse.bacc as bacc
import concourse.mybir as mybir
from concourse.bass_utils import run_bass_kernel_spmd
from concourse.tile import TileContext, add_dep_helper

F32 = mybir.dt.float32
F16 = mybir.dt.float16

N_CORES = 8
B, C, H, W = 16, 64, 256, 256
B_LOC = B // N_CORES          # 2 batches per core
N_IMG = B_LOC * C             # 128 images per core
HALF = H // 2                 # 128 rows per half
WG = W + 2                    # guarded width (258)
GROUP = int(os.environ.get("SOBEL_GROUP", "2"))   # images per tail group
HYBRID_EVERY = int(os.environ.get("SOBEL_HYBRID", "0"))  # 0 = off
DVESQ_EVERY = int(os.environ.get("SOBEL_DVESQ", "0"))    # 0 = off
FLUSH_DELAY = int(os.environ.get("SOBEL_FLUSH_DELAY", "1"))


def _tap_matrices(kern):
    """kern: [3,3]. For each horizontal tap t in {-1,0,+1} build the banded
    vertical matrix V_t[k, m] = kern[di, t+1] for k = m + di - 1 (clipped).
    Returns list of (tap, V) for taps whose column is nonzero."""
    out = []
    for t in (-1, 0, 1):
        col = kern[:, t + 1]
        if not np.any(col):
            continue
        V = np.zeros((HALF, HALF), dtype=np.float32)
        for di in range(3):
            w = float(col[di])
            if w == 0.0:
                continue
            for m in range(HALF):
                k = m + di - 1
                if 0 <= k < HALF:
                    V[k, m] = w
        out.append((t, V))
    return out


def _mm_plan(kx, ky):
    """Unique weight matrices + per-image matmul descriptors.

    Returns (mats, descs): mats = list of unique [128,128] fp32 matrices;
    descs = ordered list of (slot, bank, off, start, stop) with matmuls
    grouped by weight slot (LDWEIGHTS reuse) and start/stop flags set on
    the first/last matmul of each PSUM bank in emission order."""
    gx_taps = _tap_matrices(kx)
    gy_taps = _tap_matrices(ky)
    mats, keys = [], {}

    def slot_of(V):
        k = V.tobytes()
        if k not in keys:
            keys[k] = len(mats)
            mats.append(V)
        return keys[k]

    def finalize(raw):
        raw = sorted(raw, key=lambda d: (d[0], d[1]))
        seen_first, last_idx = set(), {}
        for j, (s, b, off) in enumerate(raw):
            last_idx[b] = j
        descs = []
        for j, (s, b, off) in enumerate(raw):
            start = b not in seen_first
            seen_first.add(b)
            descs.append((s, b, off, start, last_idx[b] == j))
        return descs

    raw = []
    for h in range(2):
        for bank, taps in ((h, gx_taps), (2 + h, gy_taps)):
            for t, V in taps:
                raw.append((slot_of(V), bank, 512 * bank + (2 - (t + 1))))
    descs = finalize(raw)

    # Hybrid "B-path" (gy via DVE/GPSIMD smooth of d = Vb x): only valid when
    # the gy taps have the separable Sobel structure v_-1 == v_+1, v_0 == 2v.
    descs_b = None
    tapmap = {t: V for t, V in gy_taps}
    if (set(tapmap) == {-1, 0, 1}
            and np.array_equal(tapmap[-1], tapmap[1])
            and np.array_equal(tapmap[0], 2 * tapmap[-1])):
        vb_slot = slot_of(tapmap[-1])
        raw_b = []
        for h in range(2):
            for t, V in gx_taps:
                raw_b.append((slot_of(V), h, 512 * h + (2 - (t + 1))))
            raw_b.append((vb_slot, 2 + h, 512 * (2 + h)))
        descs_b = finalize(raw_b)
    return mats, descs, descs_b


def _build(nc, kx, ky):
    """Trace the bass program. kx, ky: 3x3 numpy Sobel kernels."""
    x_d = nc.dram_tensor("x", [B_LOC, C, H, W], F32, kind="ExternalInput")
    w_d = nc.dram_tensor("wts", [5, HALF, HALF], F16, kind="ExternalInput")
    out_d = nc.dram_tensor("out", [B_LOC, C, H, W], F32, kind="ExternalOutput")

    _mats, mm_descs, mm_descs_b = _mm_plan(kx, ky)

    x_flat = x_d[:].rearrange("b c h w -> (b c) h w")
    out_flat = out_d[:].rearrange("b c h w -> (b c) h w")

    out_dmas = []

    with ExitStack() as ctx:
        tc = ctx.enter_context(TileContext(nc))
        wpool = ctx.enter_context(tc.tile_pool(name="wts", bufs=1))
        xpool = ctx.enter_context(tc.tile_pool(name="xin", bufs=8))
        x16pool = ctx.enter_context(tc.tile_pool(name="x16", bufs=8))
        pspool = ctx.enter_context(tc.tile_pool(name="ps", bufs=2, space="PSUM"))
        qpool = ctx.enter_context(tc.tile_pool(name="qg", bufs=int(os.environ.get("SOBEL_QBUFS", "3"))))
        mpool = ctx.enter_context(tc.tile_pool(name="mg", bufs=3))
        opool = ctx.enter_context(tc.tile_pool(name="og", bufs=3))
        spool = ctx.enter_context(tc.tile_pool(name="seam", bufs=1))
        dpool = ctx.enter_context(tc.tile_pool(name="dsb", bufs=2))
        cpool = ctx.enter_context(tc.tile_pool(name="gxc", bufs=2))
        gypool = ctx.enter_context(tc.tile_pool(name="gyb", bufs=2))

        wt = wpool.tile([HALF, 5 * HALF], F16)
        nc.sync.dma_start(
            wt[:].rearrange("k (n m) -> k n m", n=5),
            w_d[:].rearrange("n k m -> k n m"),
        )

        def flush_m(q_g, m_g, pair):
            # m = gx^2 + gy^2 for one image pair on DVE (idle engine)
            qq = q_g[:].rearrange("p (i s c) -> p i s c", i=GROUP, s=2)
            nc.vector.tensor_tensor(
                m_g[:].rearrange("p (i c) -> p i c", i=GROUP)[
                    :, 2 * pair:2 * pair + 2, :],
                qq[:, 2 * pair:2 * pair + 2, 0, :],
                qq[:, 2 * pair:2 * pair + 2, 1, :], mybir.AluOpType.add,
            )

        def flush_tail(g, m_g):
            # sqrt + store for a whole group. Emitted late so the sqrt never
            # head-of-line-blocks the PSUM-recycling squares in ACT's queue.
            o_g = opool.tile([128, GROUP * 512], F32)
            nc.scalar.activation(o_g[:], m_g[:], mybir.ActivationFunctionType.Sqrt)
            d = nc.sync.dma_start(
                out_flat[g * GROUP:(g + 1) * GROUP].rearrange(
                    "i (h p) w -> p i h w", p=128
                ),
                o_g[:].rearrange("p (i h w) -> p i h w", i=GROUP, h=2),
            )
            out_dmas.append(d)

        # ---- late seam pass, part 1: computation emitted as small steps
        # spread across the main loop so it soaks up idle engine time ----
        sx = spool.tile([128, 4 * WG], F32)   # rows 126..129, guarded
        sxv = sx[:].rearrange("p (r c) -> p r c", r=4)
        seam_steps = []

        def _seam_gather():
            nc.gpsimd.memset(sxv[:, :, 0:WG:WG - 1], 0.0)
            nc.sync.dma_start(
                sxv[:, :, 1:W + 1], x_flat[:, H // 2 - 2:H // 2 + 2, :]
            )

        seam_steps.append(_seam_gather)

        def vcomb(name, col):
            """v[r] = sum_di col[di] * x[r + di - 1] for output block rows
            1..2 (image rows 127, 128), guarded width."""
            t = spool.tile([128, 2 * WG], F32, tag=f"v_{name}")
            tv = t[:].rearrange("p (r c) -> p r c", r=2)
            up, ce, dn = sxv[:, 0:2, :], sxv[:, 1:3, :], sxv[:, 2:4, :]
            tmp = spool.tile([128, 2 * WG], F32, tag=f"vt_{name}")
            tmpv = tmp[:].rearrange("p (r c) -> p r c", r=2)

            def _s1():
                nc.vector.tensor_scalar(tmpv[:], up, float(col[0]), None,
                                        mybir.AluOpType.mult)

            def _s2():
                nc.vector.scalar_tensor_tensor(
                    tmpv[:], ce, float(col[1]), tmpv[:],
                    mybir.AluOpType.mult, mybir.AluOpType.add)

            def _s3():
                nc.vector.scalar_tensor_tensor(
                    tv[:], dn, float(col[2]), tmpv[:],
                    mybir.AluOpType.mult, mybir.AluOpType.add)

            seam_steps.extend([_s1, _s2, _s3])
            return tv

        def hcomb(name, vs):
            """sum_t vs[t] shifted by t over data cols -> [128, 2, W]"""
            ot = spool.tile([128, 2 * W], F32, tag=f"h_{name}")
            otv = ot[:].rearrange("p (r c) -> p r c", r=2)
            items = sorted(vs.items())
            acc = None
            for i, (t, tv) in enumerate(items):
                sh = tv[:, :, 1 + t:1 + t + W]
                if acc is None:
                    if len(items) == 1:
                        seam_steps.append(
                            lambda o=otv, s=sh: nc.vector.tensor_copy(o[:], s))
                    acc = sh
                elif i == len(items) - 1:
                    seam_steps.append(
                        lambda o=otv, a=acc, s=sh:
                        nc.vector.tensor_tensor(o[:], a, s, mybir.AluOpType.add))
                else:
                    t2 = spool.tile([128, 2 * W], F32, tag=f"ha_{name}_{i}")
                    t2v = t2[:].rearrange("p (r c) -> p r c", r=2)
                    seam_steps.append(
                        lambda o=t2v, a=acc, s=sh:
                        nc.vector.tensor_tensor(o[:], a, s, mybir.AluOpType.add))
                    acc = t2v[:]
            return otv

        kxc = [[float(kx[di, t]) for di in range(3)] for t in range(3)]
        kyc = [[float(ky[di, t]) for di in range(3)] for t in range(3)]
        vgx = {t: vcomb(f"gx{t}", kxc[t + 1]) for t in (-1, 0, 1)
               if any(kxc[t + 1])}
        vgy = {t: vcomb(f"gy{t}", kyc[t + 1]) for t in (-1, 0, 1)
               if any(kyc[t + 1])}
        gxs = hcomb("gx", vgx)
        gys = hcomb("gy", vgy)
        q1s = spool.tile([128, 2 * W], F32)
        q2s = spool.tile([128, 2 * W], F32)
        ms = spool.tile([128, 2 * W], F32)
        os_ = spool.tile([128, 2 * W], F32)
        seam_steps.append(lambda: nc.scalar.activation(
            q1s[:], gxs, mybir.ActivationFunctionType.Square))
        seam_steps.append(lambda: nc.scalar.activation(
            q2s[:], gys, mybir.ActivationFunctionType.Square))
        seam_steps.append(lambda: nc.vector.tensor_tensor(
            ms[:], q1s[:], q2s[:], mybir.AluOpType.add))
        seam_steps.append(lambda: nc.scalar.activation(
            os_[:], ms[:], mybir.ActivationFunctionType.Sqrt))

        n_groups = N_IMG // GROUP
        pend = []
        for g in range(n_groups):
            q_g = qpool.tile([128, GROUP * 1024], F32)
            m_g = mpool.tile([128, GROUP * 512], F32)
            for gi in range(GROUP):
                img = g * GROUP + gi
                xin = xpool.tile([128, 2 * W], F32)
                nc.sync.dma_start(
                    xin[:].rearrange("p (h w) -> p h w", h=2),
                    x_flat[img].rearrange("(h p) w -> p h w", p=128),
                )
                x16 = x16pool.tile([128, 2 * WG], F16)
                x16v = x16[:].rearrange("p (h c) -> p h c", h=2)
                # zero the 4 guard columns (robust to slot rotation), then
                # convert the data columns fp32 -> fp16 on DVE
                nc.gpsimd.memset(x16v[:, :, 0:WG:WG - 1], 0.0)
                nc.vector.tensor_copy(
                    x16v[:, :, 1:W + 1],
                    xin[:].rearrange("p (h w) -> p h w", h=2),
                )
                # 4 PSUM banks: gx-h0 | gx-h1 | gy-h0 | gy-h1 (A path)
                # or gx-h0 | gx-h1 | d-h0 | d-h1 (B path: gy on DVE/GPSIMD)
                use_b = (mm_descs_b is not None and HYBRID_EVERY > 0
                         and img % HYBRID_EVERY == 0)
                ps = pspool.tile([128, 2048], F32)
                for wslot, b, off, start, stop in (
                        mm_descs_b if use_b else mm_descs):
                    nc.tensor.matmul(
                        ps[:, off:off + WG],
                        wt[:, wslot * HALF:(wslot + 1) * HALF],
                        x16[:, (b % 2) * WG:((b % 2) + 1) * WG],
                        start=start,
                        stop=stop,
                        skip_group_check=True,
                    )
                psb = ps[:].rearrange("p (b c) -> p b c", b=4)
                qv = q_g[:].rearrange("p (i b c) -> p (i b) c", i=GROUP, b=4)
                use_c = (not use_b and DVESQ_EVERY > 0
                         and img % DVESQ_EVERY == DVESQ_EVERY - 1)
                if use_c:
                    # gy^2 on ScalarE; gx evacuated + squared on DVE
                    nc.scalar.activation(
                        qv[:, gi * 4 + 2:gi * 4 + 4, :], psb[:, 2:4, 2:W + 2],
                        mybir.ActivationFunctionType.Square,
                    )
                    gxc = cpool.tile([128, 2 * W], F32)
                    gxv = gxc[:].rearrange("p (h c) -> p h c", h=2)
                    nc.vector.tensor_copy(gxv[:], psb[:, 0:2, 2:W + 2])
                    nc.vector.tensor_tensor(
                        qv[:, gi * 4:gi * 4 + 2, :], gxv[:], gxv[:],
                        mybir.AluOpType.mult)
                elif not use_b:
                    # q = (gx|gy)^2, all 4 banks in one ScalarE op
                    nc.scalar.activation(
                        qv[:, gi * 4:(gi + 1) * 4, :], psb[:, :, 2:W + 2],
                        mybir.ActivationFunctionType.Square,
                    )
                else:
                    # gx^2 on ScalarE (banks 0-1 only)
                    nc.scalar.activation(
                        qv[:, gi * 4:gi * 4 + 2, :], psb[:, 0:2, 2:W + 2],
                        mybir.ActivationFunctionType.Square,
                    )
                    # d -> SBUF (with guard cols), u = d_l + d_r on GPSIMD,
                    # gy = 2d + u on DVE, gy^2 into q_g on GPSIMD
                    dsb = dpool.tile([128, 2 * WG], F32)
                    dv = dsb[:].rearrange("p (h c) -> p h c", h=2)
                    nc.vector.tensor_copy(dv[:], psb[:, 2:4, 0:WG])
                    u = gypool.tile([128, 2 * W], F32, tag="u")
                    uv = u[:].rearrange("p (h c) -> p h c", h=2)
                    nc.gpsimd.tensor_tensor(
                        uv[:], dv[:, :, 0:W], dv[:, :, 2:W + 2],
                        mybir.AluOpType.add)
                    gy = gypool.tile([128, 2 * W], F32, tag="gy")
                    gyv = gy[:].rearrange("p (h c) -> p h c", h=2)
                    nc.vector.scalar_tensor_tensor(
                        gyv[:], dv[:, :, 1:W + 1], 2.0, uv[:],
                        mybir.AluOpType.mult, mybir.AluOpType.add)
                    nc.gpsimd.tensor_tensor(
                        qv[:, gi * 4 + 2:gi * 4 + 4, :], gyv[:], gyv[:],
                        mybir.AluOpType.mult)
                if gi % 2 == 1:
                    flush_m(q_g, m_g, gi // 2)
            pend.append((g, m_g))
            if len(pend) > FLUSH_DELAY:
                flush_tail(*pend.pop(0))
            if g >= 3 and seam_steps:
                seam_steps.pop(0)()
        while pend:
            flush_tail(*pend.pop(0))
        while seam_steps:
            seam_steps.pop(0)()

        seam_dma = nc.sync.dma_start(
            out_flat[:, H // 2 - 1:H // 2 + 1, :],
            os_[:].rearrange("p (r c) -> p r c", r=2),
        )
        # the seam scatter must land after the bulk output DMAs
        for d in out_dmas:
            try:
                add_dep_helper(seam_dma.ins, d.ins, reason="seam after bulk out")
            except Exception:
                pass
    return nc


def _make_weights(kx, ky):
    mats, _descs, _descs_b = _mm_plan(kx, ky)
    w = np.zeros((5, HALF, HALF), dtype=np.float16)
    for i, V in enumerate(mats):
        w[i] = V.astype(np.float16)
    return w


def kernel(x, sobel_x, sobel_y):
    x = np.asarray(x)
    kx = np.asarray(sobel_x).reshape(3, 3).astype(np.float32)
    ky = np.asarray(sobel_y).reshape(3, 3).astype(np.float32)

    nc = bacc.Bacc()
    _build(nc, kx, ky)
    nc.compile()

    wts = _make_weights(kx, ky)
    in_maps = [
        {"x": np.ascontiguousarray(x[i * B_LOC:(i + 1) * B_LOC]), "wts": wts}
        for i in range(N_CORES)
    ]
    kw = {}
    if os.environ.get("BASS_SOBEL_TRACE"):
        kw = {"trace": True}
    res = run_bass_kernel_spmd(nc, in_maps, core_ids=list(range(N_CORES)), **kw)
    global LAST_RESULTS
    LAST_RESULTS = res
    return np.concatenate([r["out"] for r in res.results], axis=0)


LAST_RESULTS = None



# revision 36
# speedup vs baseline: 1.2251x; 1.2251x over previous
"""Sobel gradient magnitude kernel for Trainium2 (8 NeuronCores, batch-sharded).

out = sqrt(gx^2 + gy^2), gx/gy = 3x3 depthwise convs (zero-padded) of
x [16, 64, 256, 256] fp32.

Per-core layout (2 batches x 64 ch = 128 images of 256x256):
  - image rows on partitions, two 128-row halves side by side in the free dim
  - vertical 3-taps as banded-matrix matmuls on TensorE (fp16 in, fp32 psum)
  - horizontal taps folded into PSUM accumulation via output-shifted matmuls
  - PSUM evacuated as squares alternating between ScalarE (Square) and DVE
    (self-multiply); gx^2+gy^2 adds alternate Pool / DVE (fp16, 2x mode);
    one ScalarE Sqrt per 4-image group feeds a batched output DMA
  - input DMAs batched 2 images per transfer; weights packed [128, 5*128]
    so the weight DMA is one contiguous descriptor per partition
  - rows 127/128 of each image (cross-half seam) recomputed in one batched
    late pass over all images and scattered over the main output
"""

import os
import numpy as np
from contextlib import ExitStack

import concourse.bacc as bacc
import concourse.mybir as mybir
from concourse.bass_utils import run_bass_kernel_spmd
from concourse.tile import TileContext

F32 = mybir.dt.float32
F16 = mybir.dt.float16

N_CORES = 8
B, C, H, W = 16, 64, 256, 256
B_LOC = B // N_CORES          # 2 batches per core
N_IMG = B_LOC * C             # 128 images per core
HALF = H // 2                 # 128 rows per half
WG = W + 2                    # guarded width (258)
GROUP = int(os.environ.get("SOBEL_GROUP", "4"))   # images per in-DMA/sqrt/out-DMA
FLUSH_DELAY = int(os.environ.get("SOBEL_FLUSH_DELAY", "1"))
PREFETCH = int(os.environ.get("SOBEL_PREFETCH", "3"))     # groups of input DMA ahead
SEAM_START = int(os.environ.get("SOBEL_SEAM_START", "2"))  # group idx to start seam steps
SQRT_POS = int(os.environ.get("SOBEL_SQRT_POS", "2"))      # image slot for prev group's sqrt
EXTRA_A = int(os.environ.get("SOBEL_EXTRA_A", "16"))       # every Nth odd img -> ACT class
ADD_DVE_EVERY = int(os.environ.get("SOBEL_ADD_DVE", "0"))  # >0: every Nth A-img add on DVE


def _tap_matrices(kern):
    """kern: [3,3]. For each horizontal tap t in {-1,0,+1} build the banded
    vertical matrix V_t[k, m] = kern[di, t+1] for k = m + di - 1 (clipped).
    Returns list of (tap, V) for taps whose column is nonzero."""
    out = []
    for t in (-1, 0, 1):
        col = kern[:, t + 1]
        if not np.any(col):
            continue
        V = np.zeros((HALF, HALF), dtype=np.float32)
        for di in range(3):
            w = float(col[di])
            if w == 0.0:
                continue
            for m in range(HALF):
                k = m + di - 1
                if 0 <= k < HALF:
                    V[k, m] = w
        out.append((t, V))
    return out


def _mm_plan(kx, ky):
    """Unique weight matrices + per-image matmul descriptors.

    Returns (mats, descs): mats = list of unique [128,128] fp32 matrices;
    descs = ordered list of (slot, bank, off, start, stop) with matmuls
    grouped by weight slot (LDWEIGHTS reuse) and start/stop flags set on
    the first/last matmul of each PSUM bank in emission order."""
    gx_taps = _tap_matrices(kx)
    gy_taps = _tap_matrices(ky)
    mats, keys = [], {}

    def slot_of(V):
        k = V.tobytes()
        if k not in keys:
            keys[k] = len(mats)
            mats.append(V)
        return keys[k]

    def finalize(raw):
        raw = sorted(raw, key=lambda d: (d[0], d[1]))
        seen_first, last_idx = set(), {}
        for j, (s, b, off) in enumerate(raw):
            last_idx[b] = j
        descs = []
        for j, (s, b, off) in enumerate(raw):
            start = b not in seen_first
            seen_first.add(b)
            descs.append((s, b, off, start, last_idx[b] == j))
        return descs

    # two descriptor sets: gx (its own 2-bank PSUM tile) and gy (likewise),
    # so each half can be evacuated as soon as its matmuls finish
    raw_gx, raw_gy = [], []
    for h in range(2):
        for t, V in gx_taps:
            raw_gx.append((slot_of(V), h, 512 * h + (2 - (t + 1))))
        for t, V in gy_taps:
            raw_gy.append((slot_of(V), h, 512 * h + (2 - (t + 1))))
    return mats, finalize(raw_gx), finalize(raw_gy)


def _build(nc, kx, ky):
    """Trace the bass program. kx, ky: 3x3 numpy Sobel kernels."""
    mats, descs_gx, descs_gy = _mm_plan(kx, ky)
    n_slots = len(mats)

    x_d = nc.dram_tensor("x", [B_LOC, C, H, W], F32, kind="ExternalInput")
    w_d = nc.dram_tensor("wts", [HALF, n_slots * HALF], F16, kind="ExternalInput")
    out_d = nc.dram_tensor("out", [B_LOC, C, H, W], F32, kind="ExternalOutput")

    x_flat = x_d[:].rearrange("b c h w -> (b c) h w")
    out_flat = out_d[:].rearrange("b c h w -> (b c) h w")

    n_groups = N_IMG // GROUP

    with ExitStack() as ctx:
        tc = ctx.enter_context(TileContext(nc))
        wpool = ctx.enter_context(tc.tile_pool(name="wts", bufs=1))
        x16pool = ctx.enter_context(tc.tile_pool(name="x16", bufs=PREFETCH + 2))
        pspool = ctx.enter_context(tc.tile_pool(name="ps", bufs=2, space="PSUM"))
        qpool = ctx.enter_context(tc.tile_pool(name="qg", bufs=3))
        rpool = ctx.enter_context(tc.tile_pool(name="rg", bufs=3))
        mpool = ctx.enter_context(tc.tile_pool(name="mg", bufs=3))
        opool = ctx.enter_context(tc.tile_pool(name="og", bufs=FLUSH_DELAY + 2))
        spool = ctx.enter_context(tc.tile_pool(name="seam", bufs=1))

        wt = wpool.tile([HALF, n_slots * HALF], F16)
        nc.sync.dma_start(wt[:], w_d[:])

        # ---- input staging: Pool (SWDGE) DMA casts fp32 -> fp16 in flight,
        # writing the guarded fp16 tile directly; no separate convert pass ----
        x16_tiles = {}

        # Pre-zero the guard columns of every x16 buffer once: the cast-DMAs
        # only ever write data columns, so the guards stay zero across buffer
        # reuse and no per-group memset sits in Pool's queue.
        for _ in range(PREFETCH + 2):
            zt = x16pool.tile([128, GROUP * 2 * WG], F16, name="x16z")
            ztv = zt[:].rearrange("p (i h c) -> p i h c", i=GROUP, h=2)
            nc.gpsimd.memset(ztv[:, :, :, 0:WG:WG - 1], 0.0)

        def fetch_group(p):
            if p >= n_groups:
                return
            x16 = x16pool.tile([128, GROUP * 2 * WG], F16, name="x16z")
            x16v = x16[:].rearrange("p (i h c) -> p i h c", i=GROUP, h=2)
            # group 0 split so image 0 lands (and PE starts) sooner
            splits = ((0, 1), (1, GROUP)) if p == 0 else ((0, GROUP),)
            for lo, hi in splits:
                nc.gpsimd.dma_start(
                    x16v[:, lo:hi, :, 1:W + 1],
                    x_flat[p * GROUP + lo:p * GROUP + hi].rearrange(
                        "i (h p) w -> p i h w", p=128),
                )
            x16_tiles[p] = x16

        # ---- late seam pass: small steps spread across the main loop ----
        sx = spool.tile([128, 4 * WG], F32)   # rows 126..129, guarded
        sxv = sx[:].rearrange("p (r c) -> p r c", r=4)
        seam_steps = []

        def _seam_gather():
            nc.gpsimd.memset(sxv[:, :, 0:WG:WG - 1], 0.0)
            nc.sync.dma_start(
                sxv[:, :, 1:W + 1], x_flat[:, H // 2 - 2:H // 2 + 2, :]
            )

        seam_steps.append(_seam_gather)

        # alternate seam element-wise work between DVE and Pool
        _seam_engines = [nc.vector, nc.gpsimd]

        def _seng(j=[0]):
            j[0] ^= 1
            return _seam_engines[j[0]]

        def vcomb(name, col):
            """v[r] = sum_di col[di] * x[r + di - 1] for output block rows
            1..2 (image rows 127, 128), guarded width."""
            t = spool.tile([128, 2 * WG], F32, tag=f"v_{name}")
            tv = t[:].rearrange("p (r c) -> p r c", r=2)
            up, ce, dn = sxv[:, 0:2, :], sxv[:, 1:3, :], sxv[:, 2:4, :]
            tmp = spool.tile([128, 2 * WG], F32, tag=f"vt_{name}")
            tmpv = tmp[:].rearrange("p (r c) -> p r c", r=2)
            eng = nc.vector  # tensor_scalar/STT are DVE-only (Pool lacks them)

            def _s1():
                eng.tensor_scalar(tmpv[:], up, float(col[0]), None,
                                  mybir.AluOpType.mult)

            def _s2():
                eng.scalar_tensor_tensor(
                    tmpv[:], ce, float(col[1]), tmpv[:],
                    mybir.AluOpType.mult, mybir.AluOpType.add)

            def _s3():
                eng.scalar_tensor_tensor(
                    tv[:], dn, float(col[2]), tmpv[:],
                    mybir.AluOpType.mult, mybir.AluOpType.add)

            seam_steps.extend([_s1, _s2, _s3])
            return tv

        def hcomb(name, vs):
            """sum_t vs[t] shifted by t over data cols -> [128, 2, W]"""
            ot = spool.tile([128, 2 * W], F32, tag=f"h_{name}")
            otv = ot[:].rearrange("p (r c) -> p r c", r=2)
            items = sorted(vs.items())
            acc = None
            for i, (t, tv) in enumerate(items):
                sh = tv[:, :, 1 + t:1 + t + W]
                if acc is None:
                    if len(items) == 1:
                        seam_steps.append(
                            lambda o=otv, s=sh, e=_seng(): e.tensor_copy(o[:], s))
                    acc = sh
                elif i == len(items) - 1:
                    seam_steps.append(
                        lambda o=otv, a=acc, s=sh, e=_seng():
                        e.tensor_tensor(o[:], a, s, mybir.AluOpType.add))
                else:
                    t2 = spool.tile([128, 2 * W], F32, tag=f"ha_{name}_{i}")
                    t2v = t2[:].rearrange("p (r c) -> p r c", r=2)
                    seam_steps.append(
                        lambda o=t2v, a=acc, s=sh, e=_seng():
                        e.tensor_tensor(o[:], a, s, mybir.AluOpType.add))
                    acc = t2v[:]
            return otv

        kxc = [[float(kx[di, t]) for di in range(3)] for t in range(3)]
        kyc = [[float(ky[di, t]) for di in range(3)] for t in range(3)]
        vgx = {t: vcomb(f"gx{t}", kxc[t + 1]) for t in (-1, 0, 1)
               if any(kxc[t + 1])}
        vgy = {t: vcomb(f"gy{t}", kyc[t + 1]) for t in (-1, 0, 1)
               if any(kyc[t + 1])}
        gxs = hcomb("gx", vgx)
        gys = hcomb("gy", vgy)
        q1s = spool.tile([128, 2 * W], F32)
        q2s = spool.tile([128, 2 * W], F32)
        ms = spool.tile([128, 2 * W], F32)
        os_ = spool.tile([128, 2 * W], F32)
        seam_steps.append(lambda: nc.vector.tensor_tensor(
            q1s[:], gxs, gxs, mybir.AluOpType.mult))
        seam_steps.append(lambda: nc.gpsimd.tensor_tensor(
            q2s[:], gys, gys, mybir.AluOpType.mult))
        seam_steps.append(lambda: nc.vector.tensor_tensor(
            ms[:], q1s[:], q2s[:], mybir.AluOpType.add))
        seam_steps.append(lambda: nc.scalar.activation(
            os_[:], ms[:], mybir.ActivationFunctionType.Sqrt))
        # bulk stores skip rows 127/128, so the seam scatter can fire as
        # soon as it is ready — no ordering dependency on the output DMAs
        seam_steps.append(lambda: nc.sync.dma_start(
            out_flat[:, H // 2 - 1:H // 2 + 1, :],
            os_[:].rearrange("p (r c) -> p r c", r=2),
        ))

        def flush_sqrt(g, m_g):
            # sqrt right after the group's adds: it completes long before the
            # (delayed) output DMA reaches the head of SP's queue, so the
            # DMA's sem wait never stalls the sequencer.
            o_g = opool.tile([128, GROUP * 512], F32)
            nc.scalar.activation(o_g[:], m_g[:], mybir.ActivationFunctionType.Sqrt)
            return o_g

        def flush_dma(g, o_g):
            # two DMAs that *skip* image rows 127/128 (the cross-half seam,
            # rewritten later by the seam pass): h=0 rows live on partitions
            # 0..126, h=1 rows on partitions 1..127. This frees the seam
            # scatter from any ordering dependency on the bulk stores.
            ov = o_g[:].rearrange("p (i h w) -> p i h w", i=GROUP, h=2)
            ob = out_flat[g * GROUP:(g + 1) * GROUP].rearrange(
                "i (h p) w -> p i h w", p=128)
            nc.sync.dma_start(ob[0:127, :, 0:1, :], ov[0:127, :, 0:1, :])
            nc.sync.dma_start(ob[1:128, :, 1:2, :], ov[1:128, :, 1:2, :])

        # prologue: prefetch ahead
        for p in range(PREFETCH):
            fetch_group(p)

        pend = []
        sqrt_pend = []

        def pop_sqrt():
            pg, pm = sqrt_pend.pop(0)
            pend.append((pg, flush_sqrt(pg, pm)))
            if len(pend) > FLUSH_DELAY:
                flush_dma(*pend.pop(0))

        with nc.allow_low_precision(reason="fp16 g^2 staging; |g|<~30, tol 2e-2"):
            for g in range(n_groups):
                fetch_group(g + PREFETCH)
                m_g = mpool.tile([128, GROUP * 512], F16)
                mv = m_g[:].rearrange("p (i h c) -> p i h c", i=GROUP, h=2)
                x16 = x16_tiles.pop(g)
                q_g = qpool.tile([128, GROUP * 4 * W], F16)
                qv = q_g[:].rearrange("p (i b c) -> p i b c", i=GROUP, b=4)
                for i in range(GROUP):
                    if i == SQRT_POS and sqrt_pend:
                        # sqrt for the previous group, emitted a couple of
                        # images late so its sem wait (on that group's last
                        # add) is already satisfied when ACT's sequencer
                        # reaches it — no head-of-line block of the evacs
                        pop_sqrt()
                    img = g * GROUP + i
                    # class A: ACT Square evacuates; class B: DVE copies the
                    # raw gradients out as fp16, then squares in the fp16 2x
                    # mode (hardware allows at most one PSUM operand per op)
                    cls_a = img % 2 == 0 or (EXTRA_A and
                                             (img // 2) % EXTRA_A == EXTRA_A - 1)
                    if not cls_a:
                        r = rpool.tile([128, 4 * W], F16)
                        rv = r[:].rearrange("p (b c) -> p b c", b=4)
                    # separate 2-bank PSUM tiles for gx and gy: each half is
                    # evacuated as soon as its matmuls retire, so PE's bank
                    # recycle never waits on a full-image evacuation
                    for hi, descs in ((0, descs_gx), (1, descs_gy)):
                        ps = pspool.tile([128, 1024], F32, tag=f"ps{hi}")
                        for wslot, b, off, start, stop in descs:
                            nc.tensor.matmul(
                                ps[:, off:off + WG],
                                wt[:, wslot * HALF:(wslot + 1) * HALF],
                                x16[:, (i * 2 + b) * WG:(i * 2 + b + 1) * WG],
                                start=start,
                                stop=stop,
                                skip_group_check=True,
                            )
                        psb = ps[:].rearrange("p (b c) -> p b c", b=2)
                        if cls_a:
                            # ScalarE: q = g^2 (PSUM evac + square)
                            nc.scalar.activation(
                                qv[:, i, 2 * hi:2 * hi + 2, :],
                                psb[:, :, 2:W + 2],
                                mybir.ActivationFunctionType.Square,
                            )
                        else:
                            # DVE: evacuate raw g as fp16, square in 2x mode
                            nc.vector.tensor_copy(
                                rv[:, 2 * hi:2 * hi + 2, :],
                                psb[:, :, 2:W + 2],
                            )
                            nc.vector.tensor_tensor(
                                qv[:, i, 2 * hi:2 * hi + 2, :],
                                rv[:, 2 * hi:2 * hi + 2, :],
                                rv[:, 2 * hi:2 * hi + 2, :],
                                mybir.AluOpType.mult,
                            )
                    # m = gx^2 + gy^2 (fp16 2x on DVE; Pool takes A-class)
                    add_eng = nc.vector
                    if cls_a and not (ADD_DVE_EVERY and
                                      img % ADD_DVE_EVERY == ADD_DVE_EVERY - 1):
                        add_eng = nc.gpsimd
                    add_eng.tensor_tensor(
                        mv[:, i, :, :],
                        qv[:, i, 0:2, :], qv[:, i, 2:4, :],
                        mybir.AluOpType.add,
                    )
                sqrt_pend.append((g, m_g))
                if g >= SEAM_START:
                    for _ in range(2):
                        if seam_steps:
                            seam_steps.pop(0)()
            while sqrt_pend:
                pop_sqrt()
            while seam_steps:
                seam_steps.pop(0)()
            while pend:
                flush_dma(*pend.pop(0))
    return nc


def _make_weights(kx, ky):
    mats, _dgx, _dgy = _mm_plan(kx, ky)
    w = np.zeros((HALF, len(mats) * HALF), dtype=np.float16)
    for i, V in enumerate(mats):
        w[:, i * HALF:(i + 1) * HALF] = V.astype(np.float16)
    return w


def kernel(x, sobel_x, sobel_y):
    x = np.asarray(x)
    kx = np.asarray(sobel_x).reshape(3, 3).astype(np.float32)
    ky = np.asarray(sobel_y).reshape(3, 3).astype(np.float32)

    nc = bacc.Bacc()
    _build(nc, kx, ky)
    nc.compile()

    wts = _make_weights(kx, ky)
    in_maps = [
        {"x": np.ascontiguousarray(x[i * B_LOC:(i + 1) * B_LOC]), "wts": wts}
        for i in range(N_CORES)
    ]
    kw = {}
    if os.environ.get("BASS_SOBEL_TRACE"):
        kw = {"trace": True}
    res = run_bass_kernel_spmd(nc, in_maps, core_ids=list(range(N_CORES)), **kw)
    global LAST_RESULTS
    LAST_RESULTS = res
    return np.concatenate([r["out"] for r in res.results], axis=0)


LAST_RESULTS = None
